# revision 6
# baseline (speedup 1.0000x reference)
"""CostVolume kernel for Trainium2 (8 NeuronCores, SPMD over the H axis).

Reference computation (B=2, C=32, H=64, W=128, maxdisp=48, D=49):
    out[:, :C, d, h, w] = x[:, :, h, w]      if w >= d else 0
    out[:, C:, d, h, w] = y[:, :, h, w - d]  if w >= d else 0
    -> out shape [B, 2C, D, H, W] float32 (~205 MB)

This is pure data movement, so the kernel is DMA-dominated.  Each core owns
an 8-row slice of H.  Host-side we zero-pad each 128-float row to 176 floats
(x rows padded at the tail, y rows padded at the head).  On-chip, both
output halves then become uniform sliding-window reads:

    left  (skewed):    OUT[0, r, j, w'] = x_ext[r, j + w']
                       = x[r, j + w']           (j + w' < 128)
                       = 0                      (j + w' >= 128)
      unskew on host:  left[d, w] = OUT[0, r, d, (w - d) mod 128]
    right (d reversed) OUT[1, r, j, w] = y_ext[r, j + w]
                       = y[r, w - (48 - j)] with the w < d region exactly 0,
                       i.e. right[d] = OUT[1, r, 48 - d]  (no fixup needed)

Variant 1: store DMAs read the sliding windows directly from the input
tiles (512 B descriptor runs).  Variant 2 (default): the Vector engine
first materializes each [128, 49*128] output plane contiguously in SBUF
(overlapped with the stores of earlier planes), so the store DMAs run with
25 KB contiguous runs on both sides - near HBM line rate.
"""

import numpy as np

B, C, H, W = 2, 32, 64, 128
MAXDISP = 48
D = MAXDISP + 1          # 49
NCORES = 8
HL = H // NCORES         # 8 rows of H per core
R = B * C * HL           # 512 rows per core
PAD = MAXDISP            # 48 floats of zero padding per row
WE = W + PAD             # 176 floats per padded row
SLOTS = R // 128         # 4 rows per SBUF partition
FREE = SLOTS * WE        # 704 floats per partition
PLANE = D * W            # 6272 floats: one (d, w) output plane per row

VARIANT = 3

_CACHE = {}


def _build_bass_v1():
    """2 load DMAs + 8 sliding-window store DMAs, no compute engines."""
    import concourse.bass as bass
    import concourse.mybir as mybir

    f32 = mybir.dt.float32
    nc = bass.Bass()

    xin = nc.declare_dram_parameter("xin", [R, WE], f32, isOutput=False)
    yin = nc.declare_dram_parameter("yin", [R, WE], f32, isOutput=False)
    out = nc.declare_dram_parameter("out", [2, R, D, W], f32, isOutput=True)

    w_s, d_s, r_s = 1, W, D * W
    half_s = R * D * W

    with (
        nc.sbuf_tensor([128, FREE], f32) as xt,
        nc.sbuf_tensor([128, FREE], f32) as yt,
        nc.semaphore("dsem") as dsem,
        nc.Block() as block,
    ):
        xt_h = xt[:].tensor
        yt_h = yt[:].tensor
        out_h = out[:].tensor

        def store_dma(eng, half, tile_h, s):
            src = bass.AP(tile_h, s * WE, [[FREE, 128], [1, D], [1, W]])
            dst = bass.AP(
                out_h,
                half * half_s + s * r_s,
                [[SLOTS * r_s, 128], [d_s, D], [w_s, W]],
            )
            eng.dma_start(out=dst, in_=src).then_inc(dsem, 16)

        @block.sync
        def _(sync):
            sync.dma_start(out=xt[:], in_=xin[:]).then_inc(dsem, 16)
            sync.dma_start(out=yt[:], in_=yin[:]).then_inc(dsem, 16)
            sync.wait_ge(dsem, 32)
            for s in range(SLOTS):
                store_dma(sync, 0, xt_h, s)
            sync.wait_ge(dsem, 32 + 16 * 2 * SLOTS)

        @block.scalar
        def _(scalar):
            scalar.wait_ge(dsem, 32)
            for s in range(SLOTS):
                store_dma(scalar, 1, yt_h, s)
            scalar.wait_ge(dsem, 32 + 16 * 2 * SLOTS)

    return nc


def _build_bass_v2():
    """DVE composes contiguous planes in SBUF; stores run at line rate.

    8 chunks k = 2*s + half.  Chunk k -> compose buffer CB[k % 4].
    sync engine stores even chunks (left half), scalar odd (right half);
    vector composes, double-buffered 4 deep.
    """
    import concourse.bass as bass
    import concourse.mybir as mybir

    f32 = mybir.dt.float32
    nc = bass.Bass()

    xin = nc.declare_dram_parameter("xin", [R, WE], f32, isOutput=False)
    yin = nc.declare_dram_parameter("yin", [R, WE], f32, isOutput=False)
    out = nc.declare_dram_parameter("out", [2, R, D, W], f32, isOutput=True)

    d_s, r_s = W, D * W
    half_s = R * D * W
    NBUF = 4

    with (
        nc.sbuf_tensor([128, FREE], f32) as xt,
        nc.sbuf_tensor([128, FREE], f32) as yt,
        nc.sbuf_tensor([128, NBUF * PLANE], f32) as cb,
        nc.semaphore("lxsem") as lxsem,
        nc.semaphore("lysem") as lysem,
        nc.semaphore("csem") as csem,
        nc.semaphore("s0sem") as s0sem,
        nc.semaphore("s1sem") as s1sem,
        nc.Block() as block,
    ):
        xt_h = xt[:].tensor
        yt_h = yt[:].tensor
        cb_h = cb[:].tensor
        out_h = out[:].tensor

        def window_ap(tile_h, s):
            # sliding window over a padded row: [p][j:49][w:128], steps 1
            return bass.AP(tile_h, s * WE, [[FREE, 128], [1, D], [1, W]])

        def cb_ap3(k):
            return bass.AP(
                cb_h, (k % NBUF) * PLANE, [[NBUF * PLANE, 128], [W, D], [1, W]]
            )

        def store_dma(eng, k):
            half, s = k % 2, k // 2
            src = bass.AP(
                cb_h, (k % NBUF) * PLANE, [[NBUF * PLANE, 128], [1, PLANE]]
            )
            dst = bass.AP(
                out_h,
                half * half_s + s * r_s,
                [[SLOTS * r_s, 128], [d_s, D], [1, W]],
            )
            return eng.dma_start(out=dst, in_=src)

        @block.sync
        def _(sync):
            sync.dma_start(out=xt[:], in_=xin[:]).then_inc(lxsem, 16)
            sync.dma_start(out=yt[:], in_=yin[:]).then_inc(lysem, 16)
            for k in (0, 2, 4, 6):
                sync.wait_ge(csem, k + 1)
                store_dma(sync, k).then_inc(s0sem, 16)
            sync.wait_ge(s0sem, 64)
            sync.wait_ge(s1sem, 64)

        @block.scalar
        def _(scalar):
            for k in (1, 3, 5, 7):
                scalar.wait_ge(csem, k + 1)
                store_dma(scalar, k).then_inc(s1sem, 16)
            scalar.wait_ge(s1sem, 64)

        @block.vector
        def _(vector):
            for k in range(8):
                half, s = k % 2, k // 2
                vector.wait_ge(lxsem if half == 0 else lysem, 16)
                if k >= NBUF:
                    # buffer reuse: wait for the store of chunk k - NBUF
                    sem = s0sem if (k - NBUF) % 2 == 0 else s1sem
                    vector.wait_ge(sem, 16 * ((k - NBUF) // 2 + 1))
                tile_h = xt_h if half == 0 else yt_h
                vector.tensor_copy(out=cb_ap3(k), in_=window_ap(tile_h, s)).then_inc(
                    csem, 1
                )

    return nc


def _build_bass_v3():
    """Like v2 but with 16 half-plane chunks and composes split across the
    Vector (left half) and GpSimd (right half) engines, so stores start
    ~7 us earlier and are never compose-gated mid-stream.

    Per half: chunks i = 2*s + g, s in 0..3, g in 0..1 covering disparity
    rows [25*g, 25*g + Dg) with Dg = 25 (g=0) / 24 (g=1).
    """
    import concourse.bass as bass
    import concourse.mybir as mybir

    f32 = mybir.dt.float32
    nc = bass.Bass()

    xin = nc.declare_dram_parameter("xin", [R, WE], f32, isOutput=False)
    yin = nc.declare_dram_parameter("yin", [R, WE], f32, isOutput=False)
    out = nc.declare_dram_parameter("out", [2, R, D, W], f32, isOutput=True)

    r_s = D * W
    half_s = R * D * W
    NBUF = 4
    G0 = 25                      # disparity rows in chunk g=0
    CB = G0 * W                  # compose buffer slot: 3200 floats

    with (
        nc.sbuf_tensor([128, FREE], f32) as xt,
        nc.sbuf_tensor([128, FREE], f32) as yt,
        nc.sbuf_tensor([128, NBUF * CB], f32) as lb,
        nc.sbuf_tensor([128, NBUF * CB], f32) as rb,
        nc.semaphore("lxsem") as lxsem,
        nc.semaphore("lysem") as lysem,
        nc.semaphore("cLsem") as cLsem,
        nc.semaphore("cRsem") as cRsem,
        nc.semaphore("sLsem") as sLsem,
        nc.semaphore("sRsem") as sRsem,
        nc.Block() as block,
    ):
        xt_h = xt[:].tensor
        yt_h = yt[:].tensor
        lb_h = lb[:].tensor
        rb_h = rb[:].tensor
        out_h = out[:].tensor

        def chunk(i):
            s, g = i // 2, i % 2
            dg = G0 if g == 0 else D - G0
            return s, g, dg

        def compose(eng, tile_h, buf_h, i):
            s, g, dg = chunk(i)
            src = bass.AP(tile_h, s * WE + g * G0, [[FREE, 128], [1, dg], [1, W]])
            dst = bass.AP(
                buf_h, (i % NBUF) * CB, [[NBUF * CB, 128], [W, dg], [1, W]]
            )
            return eng.tensor_copy(out=dst, in_=src)

        def store(eng, buf_h, half, i):
            s, g, dg = chunk(i)
            src = bass.AP(buf_h, (i % NBUF) * CB, [[NBUF * CB, 128], [1, dg * W]])
            dst = bass.AP(
                out_h,
                half * half_s + s * r_s + g * G0 * W,
                [[SLOTS * r_s, 128], [1, dg * W]],
            )
            return eng.dma_start(out=dst, in_=src)

        @block.sync
        def _(sync):
            sync.dma_start(out=xt[:], in_=xin[:]).then_inc(lxsem, 16)
            sync.dma_start(out=yt[:], in_=yin[:]).then_inc(lysem, 16)
            for i in range(8):
                sync.wait_ge(cLsem, i + 1)
                store(sync, lb_h, 0, i).then_inc(sLsem, 16)
            sync.wait_ge(sLsem, 128)
            sync.wait_ge(sRsem, 128)

        @block.scalar
        def _(scalar):
            for i in range(8):
                scalar.wait_ge(cRsem, i + 1)
                store(scalar, rb_h, 1, i).then_inc(sRsem, 16)
            scalar.wait_ge(sRsem, 128)

        @block.vector
        def _(vector):
            vector.wait_ge(lxsem, 16)
            for i in range(8):
                if i >= NBUF:
                    vector.wait_ge(sLsem, 16 * (i - NBUF + 1))
                compose(vector, xt_h, lb_h, i).then_inc(cLsem, 1)

        @block.gpsimd
        def _(gpsimd):
            gpsimd.wait_ge(lysem, 16)
            for i in range(8):
                if i >= NBUF:
                    gpsimd.wait_ge(sRsem, 16 * (i - NBUF + 1))
                compose(gpsimd, yt_h, rb_h, i).then_inc(cRsem, 1)

    return nc


def _build_bass(variant):
    key = ("nc", variant)
    if key not in _CACHE:
        builders = {1: _build_bass_v1, 2: _build_bass_v2, 3: _build_bass_v3}
        _CACHE[key] = builders[variant]()
    return _CACHE[key]


def _run_on_hw(x, y, trace=False, variant=VARIANT, **trace_kwargs):
    """Shard, run the Bass kernel on 8 cores, return (per-core outs, results)."""
    from concourse.bass_utils import run_bass_kernel_spmd

    nc = _build_bass(variant)
    in_maps = []
    for k in range(NCORES):
        xk = x[:, :, HL * k : HL * (k + 1), :].reshape(R, W)
        yk = y[:, :, HL * k : HL * (k + 1), :].reshape(R, W)
        x_ext = np.zeros((R, WE), np.float32)
        x_ext[:, :W] = xk
        y_ext = np.zeros((R, WE), np.float32)
        y_ext[:, PAD:] = yk
        in_maps.append({"xin": x_ext, "yin": y_ext})

    res = run_bass_kernel_spmd(
        nc, in_maps, list(range(NCORES)), trace=trace, **trace_kwargs
    )
    return [r["out"] for r in res.results], res


def _assemble(outs):
    """Gather per-core skewed outputs into the full [B, 2C, D, H, W] array."""
    full = np.empty((B, 2 * C, D, H, W), np.float32)
    for k, oc in enumerate(outs):
        oc = oc.reshape(2, B, C, HL, D, W)
        hs = slice(HL * k, HL * (k + 1))
        # left: unskew with a per-d roll (tail of each skewed row is zeros)
        ls = oc[0].transpose(0, 1, 3, 2, 4)          # [b, c, d, h, w']
        for d in range(D):
            full[:, :C, d, hs, d:] = ls[:, :, d, :, : W - d]
            full[:, :C, d, hs, :d] = ls[:, :, d, :, W - d :]
        # right: exact, just reverse the d axis
        full[:, C:, :, hs, :] = oc[1].transpose(0, 1, 3, 2, 4)[:, :, ::-1]
    return full


def kernel(x, y, maxdisp):
    x = np.ascontiguousarray(np.asarray(x), dtype=np.float32)
    y = np.ascontiguousarray(np.asarray(y), dtype=np.float32)
    assert x.shape == (B, C, H, W) and y.shape == (B, C, H, W)
    assert int(maxdisp) == MAXDISP
    outs, _ = _run_on_hw(x, y)
    return _assemble(outs)


# revision 8
# speedup vs baseline: 1.4410x; 1.4410x over previous
"""CostVolume kernel for Trainium2 (8 NeuronCores, SPMD over the H axis).

Reference computation (B=2, C=32, H=64, W=128, maxdisp=48, D=49):
    out[:, :C, d, h, w] = x[:, :, h, w]      if w >= d else 0
    out[:, C:, d, h, w] = y[:, :, h, w - d]  if w >= d else 0
    -> out shape [B, 2C, D, H, W] float32 (~205 MB)

This is pure data movement, so the kernel is DMA-dominated.  Each core owns
an 8-row slice of H.  Host-side we zero-pad each 128-float row to 176 floats
(x rows padded at the tail, y rows padded at the head).  On-chip, both
output halves then become uniform sliding-window reads:

    left  (skewed):    OUT[0, r, j, w'] = x_ext[r, j + w']
                       = x[r, j + w']           (j + w' < 128)
                       = 0                      (j + w' >= 128)
      unskew on host:  left[d, w] = OUT[0, r, d, (w - d) mod 128]
    right (d reversed) OUT[1, r, j, w] = y_ext[r, j + w]
                       = y[r, w - (48 - j)] with the w < d region exactly 0,
                       i.e. right[d] = OUT[1, r, 48 - d]  (no fixup needed)

Variant 1: store DMAs read the sliding windows directly from the input
tiles (512 B descriptor runs).  Variant 2 (default): the Vector engine
first materializes each [128, 49*128] output plane contiguously in SBUF
(overlapped with the stores of earlier planes), so the store DMAs run with
25 KB contiguous runs on both sides - near HBM line rate.
"""

import numpy as np

B, C, H, W = 2, 32, 64, 128
MAXDISP = 48
D = MAXDISP + 1          # 49
NCORES = 8
HL = H // NCORES         # 8 rows of H per core
R = B * C * HL           # 512 rows per core
PAD = MAXDISP            # 48 floats of zero padding per row
WE = W + PAD             # 176 floats per padded row
SLOTS = R // 128         # 4 rows per SBUF partition
FREE = SLOTS * WE        # 704 floats per partition
PLANE = D * W            # 6272 floats: one (d, w) output plane per row

VARIANT = 4

_CACHE = {}


def _build_bass_v1():
    """2 load DMAs + 8 sliding-window store DMAs, no compute engines."""
    import concourse.bass as bass
    import concourse.mybir as mybir

    f32 = mybir.dt.float32
    nc = bass.Bass()

    xin = nc.declare_dram_parameter("xin", [R, WE], f32, isOutput=False)
    yin = nc.declare_dram_parameter("yin", [R, WE], f32, isOutput=False)
    out = nc.declare_dram_parameter("out", [2, R, D, W], f32, isOutput=True)

    w_s, d_s, r_s = 1, W, D * W
    half_s = R * D * W

    with (
        nc.sbuf_tensor([128, FREE], f32) as xt,
        nc.sbuf_tensor([128, FREE], f32) as yt,
        nc.semaphore("dsem") as dsem,
        nc.Block() as block,
    ):
        xt_h = xt[:].tensor
        yt_h = yt[:].tensor
        out_h = out[:].tensor

        def store_dma(eng, half, tile_h, s):
            src = bass.AP(tile_h, s * WE, [[FREE, 128], [1, D], [1, W]])
            dst = bass.AP(
                out_h,
                half * half_s + s * r_s,
                [[SLOTS * r_s, 128], [d_s, D], [w_s, W]],
            )
            eng.dma_start(out=dst, in_=src).then_inc(dsem, 16)

        @block.sync
        def _(sync):
            sync.dma_start(out=xt[:], in_=xin[:]).then_inc(dsem, 16)
            sync.dma_start(out=yt[:], in_=yin[:]).then_inc(dsem, 16)
            sync.wait_ge(dsem, 32)
            for s in range(SLOTS):
                store_dma(sync, 0, xt_h, s)
            sync.wait_ge(dsem, 32 + 16 * 2 * SLOTS)

        @block.scalar
        def _(scalar):
            scalar.wait_ge(dsem, 32)
            for s in range(SLOTS):
                store_dma(scalar, 1, yt_h, s)
            scalar.wait_ge(dsem, 32 + 16 * 2 * SLOTS)

    return nc


def _build_bass_v2():
    """DVE composes contiguous planes in SBUF; stores run at line rate.

    8 chunks k = 2*s + half.  Chunk k -> compose buffer CB[k % 4].
    sync engine stores even chunks (left half), scalar odd (right half);
    vector composes, double-buffered 4 deep.
    """
    import concourse.bass as bass
    import concourse.mybir as mybir

    f32 = mybir.dt.float32
    nc = bass.Bass()

    xin = nc.declare_dram_parameter("xin", [R, WE], f32, isOutput=False)
    yin = nc.declare_dram_parameter("yin", [R, WE], f32, isOutput=False)
    out = nc.declare_dram_parameter("out", [2, R, D, W], f32, isOutput=True)

    d_s, r_s = W, D * W
    half_s = R * D * W
    NBUF = 4

    with (
        nc.sbuf_tensor([128, FREE], f32) as xt,
        nc.sbuf_tensor([128, FREE], f32) as yt,
        nc.sbuf_tensor([128, NBUF * PLANE], f32) as cb,
        nc.semaphore("lxsem") as lxsem,
        nc.semaphore("lysem") as lysem,
        nc.semaphore("csem") as csem,
        nc.semaphore("s0sem") as s0sem,
        nc.semaphore("s1sem") as s1sem,
        nc.Block() as block,
    ):
        xt_h = xt[:].tensor
        yt_h = yt[:].tensor
        cb_h = cb[:].tensor
        out_h = out[:].tensor

        def window_ap(tile_h, s):
            # sliding window over a padded row: [p][j:49][w:128], steps 1
            return bass.AP(tile_h, s * WE, [[FREE, 128], [1, D], [1, W]])

        def cb_ap3(k):
            return bass.AP(
                cb_h, (k % NBUF) * PLANE, [[NBUF * PLANE, 128], [W, D], [1, W]]
            )

        def store_dma(eng, k):
            half, s = k % 2, k // 2
            src = bass.AP(
                cb_h, (k % NBUF) * PLANE, [[NBUF * PLANE, 128], [1, PLANE]]
            )
            dst = bass.AP(
                out_h,
                half * half_s + s * r_s,
                [[SLOTS * r_s, 128], [d_s, D], [1, W]],
            )
            return eng.dma_start(out=dst, in_=src)

        @block.sync
        def _(sync):
            sync.dma_start(out=xt[:], in_=xin[:]).then_inc(lxsem, 16)
            sync.dma_start(out=yt[:], in_=yin[:]).then_inc(lysem, 16)
            for k in (0, 2, 4, 6):
                sync.wait_ge(csem, k + 1)
                store_dma(sync, k).then_inc(s0sem, 16)
            sync.wait_ge(s0sem, 64)
            sync.wait_ge(s1sem, 64)

        @block.scalar
        def _(scalar):
            for k in (1, 3, 5, 7):
                scalar.wait_ge(csem, k + 1)
                store_dma(scalar, k).then_inc(s1sem, 16)
            scalar.wait_ge(s1sem, 64)

        @block.vector
        def _(vector):
            for k in range(8):
                half, s = k % 2, k // 2
                vector.wait_ge(lxsem if half == 0 else lysem, 16)
                if k >= NBUF:
                    # buffer reuse: wait for the store of chunk k - NBUF
                    sem = s0sem if (k - NBUF) % 2 == 0 else s1sem
                    vector.wait_ge(sem, 16 * ((k - NBUF) // 2 + 1))
                tile_h = xt_h if half == 0 else yt_h
                vector.tensor_copy(out=cb_ap3(k), in_=window_ap(tile_h, s)).then_inc(
                    csem, 1
                )

    return nc


def _build_bass_v3():
    """Like v2 but with 16 half-plane chunks and composes split across the
    Vector (left half) and GpSimd (right half) engines, so stores start
    ~7 us earlier and are never compose-gated mid-stream.

    Per half: chunks i = 2*s + g, s in 0..3, g in 0..1 covering disparity
    rows [25*g, 25*g + Dg) with Dg = 25 (g=0) / 24 (g=1).
    """
    import concourse.bass as bass
    import concourse.mybir as mybir

    f32 = mybir.dt.float32
    nc = bass.Bass()

    xin = nc.declare_dram_parameter("xin", [R, WE], f32, isOutput=False)
    yin = nc.declare_dram_parameter("yin", [R, WE], f32, isOutput=False)
    out = nc.declare_dram_parameter("out", [2, R, D, W], f32, isOutput=True)

    r_s = D * W
    half_s = R * D * W
    NBUF = 4
    G0 = 25                      # disparity rows in chunk g=0
    CB = G0 * W                  # compose buffer slot: 3200 floats

    with (
        nc.sbuf_tensor([128, FREE], f32) as xt,
        nc.sbuf_tensor([128, FREE], f32) as yt,
        nc.sbuf_tensor([128, NBUF * CB], f32) as lb,
        nc.sbuf_tensor([128, NBUF * CB], f32) as rb,
        nc.semaphore("lxsem") as lxsem,
        nc.semaphore("lysem") as lysem,
        nc.semaphore("cLsem") as cLsem,
        nc.semaphore("cRsem") as cRsem,
        nc.semaphore("sLsem") as sLsem,
        nc.semaphore("sRsem") as sRsem,
        nc.Block() as block,
    ):
        xt_h = xt[:].tensor
        yt_h = yt[:].tensor
        lb_h = lb[:].tensor
        rb_h = rb[:].tensor
        out_h = out[:].tensor

        def chunk(i):
            s, g = i // 2, i % 2
            dg = G0 if g == 0 else D - G0
            return s, g, dg

        def compose(eng, tile_h, buf_h, i):
            s, g, dg = chunk(i)
            src = bass.AP(tile_h, s * WE + g * G0, [[FREE, 128], [1, dg], [1, W]])
            dst = bass.AP(
                buf_h, (i % NBUF) * CB, [[NBUF * CB, 128], [W, dg], [1, W]]
            )
            return eng.tensor_copy(out=dst, in_=src)

        def store(eng, buf_h, half, i):
            s, g, dg = chunk(i)
            src = bass.AP(buf_h, (i % NBUF) * CB, [[NBUF * CB, 128], [1, dg * W]])
            dst = bass.AP(
                out_h,
                half * half_s + s * r_s + g * G0 * W,
                [[SLOTS * r_s, 128], [1, dg * W]],
            )
            return eng.dma_start(out=dst, in_=src)

        @block.sync
        def _(sync):
            sync.dma_start(out=xt[:], in_=xin[:]).then_inc(lxsem, 16)
            sync.dma_start(out=yt[:], in_=yin[:]).then_inc(lysem, 16)
            for i in range(8):
                sync.wait_ge(cLsem, i + 1)
                store(sync, lb_h, 0, i).then_inc(sLsem, 16)
            sync.wait_ge(sLsem, 128)
            sync.wait_ge(sRsem, 128)

        @block.scalar
        def _(scalar):
            for i in range(8):
                scalar.wait_ge(cRsem, i + 1)
                store(scalar, rb_h, 1, i).then_inc(sRsem, 16)
            scalar.wait_ge(sRsem, 128)

        @block.vector
        def _(vector):
            vector.wait_ge(lxsem, 16)
            for i in range(8):
                if i >= NBUF:
                    vector.wait_ge(sLsem, 16 * (i - NBUF + 1))
                compose(vector, xt_h, lb_h, i).then_inc(cLsem, 1)

        @block.gpsimd
        def _(gpsimd):
            gpsimd.wait_ge(lysem, 16)
            for i in range(8):
                if i >= NBUF:
                    gpsimd.wait_ge(sRsem, 16 * (i - NBUF + 1))
                compose(gpsimd, yt_h, rb_h, i).then_inc(cRsem, 1)

    return nc


def _build_bass_v4():
    """16 half-plane chunks, all composes on the Vector engine, interleaved
    left/right so both store queues fill evenly.  Chunk g=0 covers d rows
    [0, 24), g=1 covers [24, 49) - both source offsets 32B-aligned (the
    misaligned 100 B offset of the v3 split cost 2.5x on DVE copies).
    """
    import concourse.bass as bass
    import concourse.mybir as mybir

    f32 = mybir.dt.float32
    nc = bass.Bass()

    xin = nc.declare_dram_parameter("xin", [R, WE], f32, isOutput=False)
    yin = nc.declare_dram_parameter("yin", [R, WE], f32, isOutput=False)
    out = nc.declare_dram_parameter("out", [2, R, D, W], f32, isOutput=True)

    r_s = D * W
    half_s = R * D * W
    NBUF = 4
    CB = 25 * W                  # compose buffer slot: 3200 floats

    with (
        nc.sbuf_tensor([128, FREE], f32) as xt,
        nc.sbuf_tensor([128, FREE], f32) as yt,
        nc.sbuf_tensor([128, NBUF * CB], f32) as lb,
        nc.sbuf_tensor([128, NBUF * CB], f32) as rb,
        nc.semaphore("lxsem") as lxsem,
        nc.semaphore("lysem") as lysem,
        nc.semaphore("cLsem") as cLsem,
        nc.semaphore("cRsem") as cRsem,
        nc.semaphore("sLsem") as sLsem,
        nc.semaphore("sRsem") as sRsem,
        nc.Block() as block,
    ):
        xt_h = xt[:].tensor
        yt_h = yt[:].tensor
        lb_h = lb[:].tensor
        rb_h = rb[:].tensor
        out_h = out[:].tensor

        def chunk(i):
            s, g = i // 2, i % 2
            d0 = 0 if g == 0 else 24
            dg = 24 if g == 0 else 25
            return s, d0, dg

        def compose(eng, tile_h, buf_h, i):
            s, d0, dg = chunk(i)
            src = bass.AP(tile_h, s * WE + d0, [[FREE, 128], [1, dg], [1, W]])
            dst = bass.AP(
                buf_h, (i % NBUF) * CB, [[NBUF * CB, 128], [W, dg], [1, W]]
            )
            return eng.tensor_copy(out=dst, in_=src)

        def store(eng, buf_h, half, i):
            s, d0, dg = chunk(i)
            src = bass.AP(buf_h, (i % NBUF) * CB, [[NBUF * CB, 128], [1, dg * W]])
            dst = bass.AP(
                out_h,
                half * half_s + s * r_s + d0 * W,
                [[SLOTS * r_s, 128], [1, dg * W]],
            )
            return eng.dma_start(out=dst, in_=src)

        @block.sync
        def _(sync):
            sync.dma_start(out=xt[:], in_=xin[:]).then_inc(lxsem, 16)
            sync.dma_start(out=yt[:], in_=yin[:]).then_inc(lysem, 16)
            for i in range(8):
                sync.wait_ge(cLsem, i + 1)
                store(sync, lb_h, 0, i).then_inc(sLsem, 16)
            sync.wait_ge(sLsem, 128)
            sync.wait_ge(sRsem, 128)

        @block.scalar
        def _(scalar):
            for i in range(8):
                scalar.wait_ge(cRsem, i + 1)
                store(scalar, rb_h, 1, i).then_inc(sRsem, 16)
            scalar.wait_ge(sRsem, 128)

        @block.vector
        def _(vector):
            vector.wait_ge(lxsem, 16)
            for i in range(8):
                if i >= NBUF:
                    vector.wait_ge(sLsem, 16 * (i - NBUF + 1))
                compose(vector, xt_h, lb_h, i).then_inc(cLsem, 1)
                if i == 0:
                    vector.wait_ge(lysem, 16)
                if i >= NBUF:
                    vector.wait_ge(sRsem, 16 * (i - NBUF + 1))
                compose(vector, yt_h, rb_h, i).then_inc(cRsem, 1)

    return nc


def _build_bass(variant):
    key = ("nc", variant)
    if key not in _CACHE:
        builders = {
            1: _build_bass_v1,
            2: _build_bass_v2,
            3: _build_bass_v3,
            4: _build_bass_v4,
        }
        _CACHE[key] = builders[variant]()
    return _CACHE[key]


def _run_on_hw(x, y, trace=False, variant=VARIANT, **trace_kwargs):
    """Shard, run the Bass kernel on 8 cores, return (per-core outs, results)."""
    from concourse.bass_utils import run_bass_kernel_spmd

    nc = _build_bass(variant)
    in_maps = []
    for k in range(NCORES):
        xk = x[:, :, HL * k : HL * (k + 1), :].reshape(R, W)
        yk = y[:, :, HL * k : HL * (k + 1), :].reshape(R, W)
        x_ext = np.zeros((R, WE), np.float32)
        x_ext[:, :W] = xk
        y_ext = np.zeros((R, WE), np.float32)
        y_ext[:, PAD:] = yk
        in_maps.append({"xin": x_ext, "yin": y_ext})

    res = run_bass_kernel_spmd(
        nc, in_maps, list(range(NCORES)), trace=trace, **trace_kwargs
    )
    return [r["out"] for r in res.results], res


def _assemble(outs):
    """Gather per-core skewed outputs into the full [B, 2C, D, H, W] array."""
    full = np.empty((B, 2 * C, D, H, W), np.float32)
    for k, oc in enumerate(outs):
        oc = oc.reshape(2, B, C, HL, D, W)
        hs = slice(HL * k, HL * (k + 1))
        # left: unskew with a per-d roll (tail of each skewed row is zeros)
        ls = oc[0].transpose(0, 1, 3, 2, 4)          # [b, c, d, h, w']
        for d in range(D):
            full[:, :C, d, hs, d:] = ls[:, :, d, :, : W - d]
            full[:, :C, d, hs, :d] = ls[:, :, d, :, W - d :]
        # right: exact, just reverse the d axis
        full[:, C:, :, hs, :] = oc[1].transpose(0, 1, 3, 2, 4)[:, :, ::-1]
    return full


def kernel(x, y, maxdisp):
    x = np.ascontiguousarray(np.asarray(x), dtype=np.float32)
    y = np.ascontiguousarray(np.asarray(y), dtype=np.float32)
    assert x.shape == (B, C, H, W) and y.shape == (B, C, H, W)
    assert int(maxdisp) == MAXDISP
    outs, _ = _run_on_hw(x, y)
    return _assemble(outs)
